# revision 17
# baseline (speedup 1.0000x reference)
"""Trainium2 Bass kernel for nn_Attention_66546223284383.

Strategy: pure data-parallel over batch B=16 -> 2 batches per core x 8 cores.
Per core, per batch:
  qkvT = (BN-folded W)^T @ x^T           (h on partitions, n free)
  per head: scoresT = k^T.T @ qT         (m on partitions, n free; attn scale
            folded into q weights), exp on ACT (scores are small: no max-sub),
            oT = [v|1]^T @ expT          (65 rows: 64 o-dims + denominator),
            PE-transpose -> normalize -> stage o to DRAM (n, c) bf16.
  conv branch: v reflowed via DRAM to (channel, spatial), hardswish + 9-tap
            depthwise conv on VectorE with per-partition tap weights.
  proj:     O2^T tiles read back via XBAR transpose DMA (handles the torch
            "raw reshape" (B,N,H,d)->(B,DH,N) as a flat re-chunk),
            xo = O2 @ proj_w^T + vc  (r on partitions, j free)
  out:      out = xo^T @ out_w^T + out_b -> (1024, 512) fp32.
All matmuls bf16 inputs with fp32 PSUM accumulation.
"""
import sys
import numpy as np

sys.path.insert(0, "/opt/trn_rl_repo")

import ml_dtypes  # noqa: E402

BF16 = ml_dtypes.bfloat16

KD, H, D, DH, DIM, IMG, S, N, B = 32, 8, 64, 512, 512, 1024, 32, 1044, 16
EPS = 1e-5
NCORES = 8
BPC = B // NCORES  # batches per core
NT = [(i * 128, 128) for i in range(8)] + [(1024, 20)]   # 1044 partition tiles
NCH = [(0, 512), (512, 512), (1024, 20)]                 # 1044 free chunks

_cached = {}


def _build():
    from concourse import bacc, tile
    import concourse.bass as bass
    import concourse.mybir as mybir
    from concourse.masks import make_identity

    dt = mybir.dt
    alu = mybir.AluOpType
    act_exp = mybir.ActivationFunctionType.Exp

    nc = bacc.Bacc(None, target_bir_lowering=False, debug=False)

    xs = nc.declare_dram_parameter("xs", [BPC, N, DIM], dt.bfloat16, isOutput=False)
    wqkv = nc.declare_dram_parameter("wqkv", [DIM, 1280], dt.bfloat16, isOutput=False)
    bqkv = nc.declare_dram_parameter("bqkv", [128, 10], dt.float32, isOutput=False)
    wproj = nc.declare_dram_parameter("wproj", [N, IMG], dt.bfloat16, isOutput=False)
    wout = nc.declare_dram_parameter("wout", [DH, DIM], dt.bfloat16, isOutput=False)
    tapw = nc.declare_dram_parameter("tapw", [128, 36], dt.float32, isOutput=False)
    cbp = nc.declare_dram_parameter("cbp", [128, 4], dt.float32, isOutput=False)
    pbp = nc.declare_dram_parameter("pbp", [1, IMG], dt.float32, isOutput=False)
    obp = nc.declare_dram_parameter("obp", [1, DIM], dt.float32, isOutput=False)
    out_ext = nc.declare_dram_parameter("out", [BPC, IMG, DIM], dt.float32, isOutput=True)

    o_nat = nc.dram_tensor("o_nat", [BPC, N, DH], dt.bfloat16)
    v_dram = nc.dram_tensor("v_dram", [BPC, H, N, D], dt.bfloat16)

    with tile.TileContext(nc) as tc:
        with (
            tc.tile_pool(name="w", bufs=1) as pw,
            tc.tile_pool(name="xT", bufs=6) as pxT,
            tc.tile_pool(name="qkvT", bufs=9) as pqk,
            tc.tile_pool(name="ve", bufs=20) as pve,
            tc.tile_pool(name="exp", bufs=3) as pexp,
            tc.tile_pool(name="oTs", bufs=2) as poTs,
            tc.tile_pool(name="small", bufs=4) as psm,
            tc.tile_pool(name="conv", bufs=2) as pcv,
            tc.tile_pool(name="cin", bufs=6) as pcin,
            tc.tile_pool(name="vc", bufs=8) as pvc,
            tc.tile_pool(name="o2t", bufs=1) as po2,
            tc.tile_pool(name="xo", bufs=8) as pxo,
            tc.tile_pool(name="psmm", bufs=5, space=bass.MemorySpace.PSUM) as psmm,
            tc.tile_pool(name="psot", bufs=3, space=bass.MemorySpace.PSUM) as psot,
        ):
            # ---- constants / weights ----
            id_sb = pw.tile([128, 128], dt.bfloat16, tag="id")
            make_identity(nc, id_sb[:])
            wqkv_sb = pw.tile([128, 4, 1280], dt.bfloat16, tag="wqkv")
            nc.sync.dma_start(wqkv_sb[:], wqkv[:].rearrange("(k p) h -> p k h", p=128))
            bqkv_sb = pw.tile([128, 10], dt.float32, tag="bqkv")
            nc.sync.dma_start(bqkv_sb[:], bqkv[:])
            wproj_sb = []
            for mt, (m0, msz) in enumerate(NT):
                t = pw.tile([msz, 1024], dt.bfloat16, tag=f"wproj{mt}")
                nc.sync.dma_start(t[:], wproj[m0:m0 + msz, :])
                wproj_sb.append(t)
            wout_sb = pw.tile([128, 4, DIM], dt.bfloat16, tag="wout")
            nc.sync.dma_start(wout_sb[:], wout[:].rearrange("(k p) c -> p k c", p=128))
            tapw_sb = pw.tile([128, 36], dt.float32, tag="tapw")
            nc.sync.dma_start(tapw_sb[:], tapw[:])
            cb_sb = pw.tile([128, 4], dt.float32, tag="cb")
            nc.sync.dma_start(cb_sb[:], cbp[:])
            pb_sb = pw.tile([1, IMG], dt.float32, tag="pb")
            nc.sync.dma_start(pb_sb[:], pbp[:])
            ob_sb = pw.tile([1, DIM], dt.float32, tag="ob")
            nc.sync.dma_start(ob_sb[:], obp[:])
            pbbc = pw.tile([128, IMG], dt.float32, tag="pbbc")
            nc.gpsimd.partition_broadcast(pbbc[:], pb_sb[:])
            obbc = pw.tile([128, DIM], dt.float32, tag="obbc")
            nc.gpsimd.partition_broadcast(obbc[:], ob_sb[:])

            for b in range(BPC):
                # ---- x^T via XBAR transpose DMA ----
                xT = []
                for cb4 in range(4):
                    t = pxT.tile([128, N], dt.bfloat16, tag="xT")
                    c0 = cb4 * 128
                    nc.sync.dma_start_transpose(t[:, 0:1040], xs[b, 0:1040, c0:c0 + 128])
                    nc.sync.dma_start(
                        t[:, 1040:N], xs[b, 1040:N, c0:c0 + 128].rearrange("a b -> b a")
                    )
                    xT.append(t)

                # ---- qkv matmul + BN bias, qkvT (h on partitions) ----
                # 10 padded tiles: 0-2 q heads (3/tile @ 0,32,64), 3-5 k heads,
                # 6-9 v heads (2/tile @ 0,64)
                qkvT = []
                for mt10 in range(10):
                    t = pqk.tile([128, N], dt.bfloat16, tag="qkvT")
                    for (ci, cw) in NCH:
                        ps = psmm.tile([128, cw], dt.float32, tag="mm")
                        for kc in range(4):
                            nc.tensor.matmul(
                                ps[:],
                                wqkv_sb[:, kc, mt10 * 128:(mt10 + 1) * 128],
                                xT[kc][:, ci:ci + cw],
                                start=(kc == 0), stop=(kc == 3),
                            )
                        nc.vector.tensor_scalar(
                            t[:, ci:ci + cw], ps[:], bqkv_sb[:, mt10:mt10 + 1], None,
                            op0=alu.add,
                        )
                    qkvT.append(t)

                # ---- per head: v transpose (+ones), stage v, attention ----
                vc_tiles = []
                for hh in range(H):
                    # q/k at base partitions {0,32,64} in padded tiles
                    qo = (hh % 3) * KD
                    qT = qkvT[hh // 3][qo:qo + KD, :]
                    kT = qkvT[3 + hh // 3][qo:qo + KD, :]
                    vo = (hh % 2) * D
                    # v^T -> v (n on partitions), add ones column
                    ves = []
                    for mt, (m0, msz) in enumerate(NT):
                        tp = psmm.tile([msz, D], dt.bfloat16, tag="mm")
                        nc.tensor.transpose(
                            tp[:], qkvT[6 + hh // 2][vo:vo + D, m0:m0 + msz],
                            id_sb[vo:vo + D, vo:vo + D],
                        )
                        ve = pve.tile([msz, D + 1], dt.bfloat16, tag="ve")
                        nc.vector.tensor_copy(ve[:, 0:D], tp[:])
                        nc.vector.memset(ve[:, D:D + 1], 1.0)
                        nc.sync.dma_start(v_dram[b, hh, m0:m0 + msz, :], ve[:, 0:D])
                        ves.append(ve)

                    # attention: scoresT/expT (m part, n free), oT accum
                    oT_ps = [
                        psot.tile([D + 1, cw], dt.float32, tag="ot", name=f"ot{k}")
                        for k, (_, cw) in enumerate(NCH)
                    ]
                    for mt, (m0, msz) in enumerate(NT):
                        et = pexp.tile([128, N], dt.bfloat16, tag="exp")
                        for (ci, cw) in NCH:
                            sc = psmm.tile([msz, cw], dt.float32, tag="mm")
                            nc.tensor.matmul(
                                sc[:], kT[:, m0:m0 + msz], qT[:, ci:ci + cw],
                                start=True, stop=True,
                            )
                            nc.scalar.activation(et[0:msz, ci:ci + cw], sc[:], act_exp)
                        for k, (ci, cw) in enumerate(NCH):
                            nc.tensor.matmul(
                                oT_ps[k][:], ves[mt][:], et[0:msz, ci:ci + cw],
                                start=(mt == 0), stop=(mt == 8),
                            )
                    oT_sb = poTs.tile([D + 1, N], dt.bfloat16, tag="oTs")
                    for k, (ci, cw) in enumerate(NCH):
                        nc.vector.tensor_copy(oT_sb[:, ci:ci + cw], oT_ps[k][:])
                    # transpose back, normalize, stage o
                    for nt, (n0, nsz) in enumerate(NT):
                        tp2 = psmm.tile([nsz, D + 1], dt.bfloat16, tag="mm")
                        nc.tensor.transpose(
                            tp2[:], oT_sb[:, n0:n0 + nsz], id_sb[0:D + 1, 0:D + 1]
                        )
                        rcp = psm.tile([nsz, 1], dt.float32, tag="rcp")
                        nc.vector.reciprocal(rcp[:], tp2[:, D:D + 1])
                        onrm = psm.tile([nsz, D], dt.bfloat16, tag="onrm")
                        nc.vector.tensor_scalar(
                            onrm[:], tp2[:, 0:D], rcp[:], None, op0=alu.mult
                        )
                        nc.sync.dma_start(
                            o_nat[b, n0:n0 + nsz, hh * D:(hh + 1) * D], onrm[:]
                        )

                    # conv branch per channel-tile once its two heads are staged
                    if hh % 2 == 1:
                        ct = hh // 2
                        cin = pcin.tile([128, 1024], dt.bfloat16, tag="cin")
                        for hl in range(2):
                            src = v_dram[b, 2 * ct + hl, 0:1024, :].rearrange(
                                "(c n2) d -> c (n2 d)", c=64
                            )
                            nc.sync.dma_start(cin[hl * 64:(hl + 1) * 64, :], src)
                        u = pcv.tile([128, 1024], dt.float32, tag="u")
                        nc.vector.tensor_scalar(
                            u[:], cin[:], 1.0 / 6.0, 0.5, op0=alu.mult, op1=alu.add
                        )
                        nc.vector.tensor_scalar(
                            u[:], u[:], 1.0, 0.0, op0=alu.min, op1=alu.max
                        )
                        hs = pcv.tile([128, 32, 32], dt.bfloat16, tag="hs")
                        nc.vector.tensor_tensor(
                            hs[:].rearrange("p a b -> p (a b)"), u[:], cin[:],
                            op=alu.mult,
                        )
                        pad = pcv.tile([128, 34, 34], dt.bfloat16, tag="pad")
                        nc.vector.memset(pad[:], 0.0)
                        nc.vector.tensor_copy(pad[:, 1:33, 1:33], hs[:])
                        acc = pcv.tile([128, 1024], dt.float32, tag="acc")
                        t2 = pcv.tile([128, 1024], dt.float32, tag="t2")
                        for tap in range(9):
                            dy, dx = tap // 3, tap % 3
                            view = pad[:, dy:dy + 32, dx:dx + 32]
                            wsl = tapw_sb[:, ct * 9 + tap:ct * 9 + tap + 1]
                            if tap == 0:
                                nc.vector.tensor_scalar(
                                    acc[:].rearrange("p (a b) -> p a b", a=32),
                                    view, wsl, None, op0=alu.mult,
                                )
                            else:
                                nc.vector.tensor_scalar(
                                    t2[:].rearrange("p (a b) -> p a b", a=32),
                                    view, wsl, None, op0=alu.mult,
                                )
                                nc.vector.tensor_tensor(acc[:], acc[:], t2[:], op=alu.add)
                        nc.vector.tensor_scalar(
                            acc[:], acc[:], cb_sb[:, ct:ct + 1], None, op0=alu.add
                        )
                        nc.vector.tensor_tensor(acc[:], acc[:], pbbc[:], op=alu.add)
                        vct = pvc.tile([128, 1024], dt.bfloat16, tag="vc")
                        nc.vector.tensor_copy(vct[:], acc[:])
                        vc_tiles.append(vct)

                # ---- proj: O2^T tiles via XBAR transpose, xo = O2 @ wproj + vc ----
                O2v = o_nat[b].rearrange("n c -> (n c)").rearrange("(r m) -> r m", m=N)
                o2t = []
                for mt, (m0, msz) in enumerate(NT):
                    t = po2.tile([msz, DH], dt.bfloat16, tag=f"o2t{mt}")
                    if msz == 128:
                        nc.sync.dma_start_transpose(t[:], O2v[:, m0:m0 + msz])
                    else:
                        nc.sync.dma_start(
                            t[:], O2v[:, m0:m0 + msz].rearrange("a b -> b a")
                        )
                    o2t.append(t)
                xo_sb = [
                    pxo.tile([128, 1024], dt.bfloat16, tag="xo", name=f"xo{rt}")
                    for rt in range(4)
                ]
                for jc in (0, 512):
                    for rt in range(4):
                        ps = psmm.tile([128, 512], dt.float32, tag="mm")
                        for mt, (m0, msz) in enumerate(NT):
                            nc.tensor.matmul(
                                ps[:],
                                o2t[mt][0:msz, rt * 128:(rt + 1) * 128],
                                wproj_sb[mt][0:msz, jc:jc + 512],
                                start=(mt == 0), stop=(mt == 8),
                            )
                        nc.vector.tensor_tensor(
                            xo_sb[rt][:, jc:jc + 512], ps[:],
                            vc_tiles[rt][:, jc:jc + 512], op=alu.add,
                        )

                # ---- out = xo^T @ wout^T + out_b ----
                for jt in range(8):
                    ps = psmm.tile([128, 512], dt.float32, tag="mm")
                    for rt in range(4):
                        nc.tensor.matmul(
                            ps[:],
                            xo_sb[rt][:, jt * 128:(jt + 1) * 128],
                            wout_sb[:, rt, :],
                            start=(rt == 0), stop=(rt == 3),
                        )
                    osb = psm.tile([128, DIM], dt.float32, tag="outsb")
                    nc.vector.tensor_tensor(osb[:], ps[:], obbc[:], op=alu.add)
                    nc.sync.dma_start(out_ext[b, jt * 128:(jt + 1) * 128, :], osb[:])

    nc.compile()
    return nc


def _prep_weights(qkv_w, bn_gamma, bn_beta, bn_mean, bn_var,
                  conv_w, conv_b, proj_w, proj_b, out_w, out_b):
    s = bn_gamma / np.sqrt(bn_var + EPS)
    bias = bn_beta - bn_mean * s
    Wt = (qkv_w * s[:, None]).T.copy()
    bvec = bias.copy()
    scale = KD ** -0.5
    for hh in range(H):
        Wt[:, hh * 128:hh * 128 + KD] *= scale
        bvec[hh * 128:hh * 128 + KD] *= scale
    # scatter channels into padded 1280 layout (see _build head slicing):
    # tiles 0-2: q heads 3/tile at offsets 0,32,64; 3-5: k; 6-9: v 2/tile at 0,64
    Wn = np.zeros((DIM, 1280), Wt.dtype)
    bn = np.zeros(1280, bvec.dtype)
    for hh in range(H):
        qdst = (hh // 3) * 128 + (hh % 3) * 32
        kdst = 384 + (hh // 3) * 128 + (hh % 3) * 32
        vdst = 768 + (hh // 2) * 128 + (hh % 2) * 64
        qsrc, ksrc, vsrc = hh * 128, hh * 128 + 32, hh * 128 + 64
        Wn[:, qdst:qdst + 32] = Wt[:, qsrc:qsrc + 32]
        bn[qdst:qdst + 32] = bvec[qsrc:qsrc + 32]
        Wn[:, kdst:kdst + 32] = Wt[:, ksrc:ksrc + 32]
        bn[kdst:kdst + 32] = bvec[ksrc:ksrc + 32]
        Wn[:, vdst:vdst + 64] = Wt[:, vsrc:vsrc + 64]
        bn[vdst:vdst + 64] = bvec[vsrc:vsrc + 64]
    Wt = Wn
    bvec = bn
    return {
        "wqkv": np.ascontiguousarray(Wt).astype(BF16),
        "bqkv": np.ascontiguousarray(bvec.reshape(10, 128).T).astype(np.float32),
        "wproj": np.ascontiguousarray(proj_w.T).astype(BF16),
        "wout": np.ascontiguousarray(out_w.T).astype(BF16),
        "tapw": np.ascontiguousarray(
            conv_w[:, 0].reshape(4, 128, 9).transpose(1, 0, 2).reshape(128, 36)
        ).astype(np.float32),
        "cbp": np.ascontiguousarray(conv_b.reshape(4, 128).T).astype(np.float32),
        "pbp": proj_b.reshape(1, IMG).astype(np.float32),
        "obp": out_b.reshape(1, DIM).astype(np.float32),
    }


def run(trace=False, tmpdir=None, **inputs):
    from concourse.bass_utils import run_bass_kernel_spmd

    if "nc" not in _cached:
        _cached["nc"] = _build()
    nc = _cached["nc"]

    w = _prep_weights(**{k: np.asarray(v) for k, v in inputs.items() if k != "x"})
    x = np.asarray(inputs["x"]).astype(BF16)
    in_maps = []
    for c in range(NCORES):
        m = dict(w)
        m["xs"] = np.ascontiguousarray(x[c * BPC:(c + 1) * BPC])
        in_maps.append(m)
    res = run_bass_kernel_spmd(
        nc, in_maps, core_ids=list(range(NCORES)), trace=trace, tmpdir=tmpdir
    )
    out = np.concatenate([np.asarray(r["out"]) for r in res.results], axis=0)
    return out.astype(np.float32), res.exec_time_ns


def kernel(**inputs):
    out, _ = run(trace=False, **inputs)
    return out


if __name__ == "__main__":
    print("building graph...")
    nc = _build()
    print("build OK:", len(nc.m.functions[0].allocations), "allocations")


# revision 25
# speedup vs baseline: 1.0373x; 1.0373x over previous
"""Trainium2 Bass kernel for nn_Attention_66546223284383.

Strategy: pure data-parallel over batch B=16 -> 2 batches per core x 8 cores.
Per core, per batch:
  qkvT = (BN-folded W)^T @ x^T           (h on partitions, n free)
  per head: scoresT = k^T.T @ qT         (m on partitions, n free; attn scale
            folded into q weights), exp on ACT (scores are small: no max-sub),
            oT = [v|1]^T @ expT          (65 rows: 64 o-dims + denominator),
            PE-transpose -> normalize -> stage o to DRAM (n, c) bf16.
  conv branch: v reflowed via DRAM to (channel, spatial), hardswish + 9-tap
            depthwise conv on VectorE with per-partition tap weights.
  proj:     O2^T tiles read back via XBAR transpose DMA (handles the torch
            "raw reshape" (B,N,H,d)->(B,DH,N) as a flat re-chunk),
            xo = O2 @ proj_w^T + vc  (r on partitions, j free)
  out:      out = xo^T @ out_w^T + out_b -> (1024, 512) fp32.
All matmuls bf16 inputs with fp32 PSUM accumulation.
"""
import sys
import numpy as np

sys.path.insert(0, "/opt/trn_rl_repo")

import ml_dtypes  # noqa: E402

BF16 = ml_dtypes.bfloat16

KD, H, D, DH, DIM, IMG, S, N, B = 32, 8, 64, 512, 512, 1024, 32, 1044, 16
EPS = 1e-5
NCORES = 8
BPC = B // NCORES  # batches per core
NT = [(i * 128, 128) for i in range(8)] + [(1024, 20)]   # 1044 partition tiles
NCH = [(0, 512), (512, 512), (1024, 20)]                 # 1044 free chunks

_cached = {}


def _build():
    from concourse import bacc, tile
    import concourse.bass as bass
    import concourse.mybir as mybir
    from concourse.masks import make_identity

    dt = mybir.dt
    alu = mybir.AluOpType
    act_exp = mybir.ActivationFunctionType.Exp

    nc = bacc.Bacc(None, target_bir_lowering=False, debug=False)

    xs = nc.declare_dram_parameter("xs", [BPC, N, DIM], dt.bfloat16, isOutput=False)
    # wqkv cols: 0:768 = 6 padded qk tiles (3 q tiles then 3 k tiles, 3 heads
    # per tile at offsets 0/32/64); 768:1288 = v channels hh*65+j (j==64 is a
    # ones channel: zero weights, bias 1 -> softmax denominator column)
    wqkv = nc.declare_dram_parameter("wqkv", [DIM, 1288], dt.bfloat16, isOutput=False)
    bqkv = nc.declare_dram_parameter("bqkv", [128, 6], dt.float32, isOutput=False)
    bv = nc.declare_dram_parameter("bv", [1, 520], dt.float32, isOutput=False)
    wproj = nc.declare_dram_parameter("wproj", [N, IMG], dt.bfloat16, isOutput=False)
    wout = nc.declare_dram_parameter("wout", [DH, DIM], dt.bfloat16, isOutput=False)
    tapw = nc.declare_dram_parameter("tapw", [128, 36], dt.float32, isOutput=False)
    cbp = nc.declare_dram_parameter("cbp", [128, 4], dt.float32, isOutput=False)
    pbp = nc.declare_dram_parameter("pbp", [1, IMG], dt.float32, isOutput=False)
    obp = nc.declare_dram_parameter("obp", [1, DIM], dt.float32, isOutput=False)
    out_ext = nc.declare_dram_parameter("out", [BPC, IMG, DIM], dt.float32, isOutput=True)

    o_nat = nc.dram_tensor("o_nat", [BPC, N, DH], dt.bfloat16)
    v_dram = nc.dram_tensor("v_dram", [BPC, H, N, D], dt.bfloat16)

    with tile.TileContext(nc) as tc:
        with (
            tc.tile_pool(name="w", bufs=1) as pw,
            tc.tile_pool(name="xT", bufs=6) as pxT,
            tc.tile_pool(name="qkvT", bufs=9) as pqk,
            tc.tile_pool(name="vnat", bufs=18) as pvn,
            tc.tile_pool(name="exp", bufs=3) as pexp,
            tc.tile_pool(name="oTs", bufs=2) as poTs,
            tc.tile_pool(name="small", bufs=4) as psm,
            tc.tile_pool(name="conv", bufs=2) as pcv,
            tc.tile_pool(name="cin", bufs=6) as pcin,
            tc.tile_pool(name="vc", bufs=8) as pvc,
            tc.tile_pool(name="o2t", bufs=1) as po2,
            tc.tile_pool(name="xo", bufs=8) as pxo,
            tc.tile_pool(name="psmm", bufs=5, space=bass.MemorySpace.PSUM) as psmm,
            tc.tile_pool(name="psot", bufs=3, space=bass.MemorySpace.PSUM) as psot,
        ):
            # ---- constants / weights ----
            id_sb = pw.tile([128, 128], dt.bfloat16, tag="id")
            make_identity(nc, id_sb[:])
            wqkv_sb = pw.tile([128, 4, 1288], dt.bfloat16, tag="wqkv")
            nc.sync.dma_start(wqkv_sb[:], wqkv[:].rearrange("(k p) h -> p k h", p=128))
            bqkv_sb = pw.tile([128, 6], dt.float32, tag="bqkv")
            nc.sync.dma_start(bqkv_sb[:], bqkv[:])
            bv_sb = pw.tile([1, 520], dt.float32, tag="bv")
            nc.sync.dma_start(bv_sb[:], bv[:])
            bvbc = pw.tile([128, 520], dt.float32, tag="bvbc")
            nc.gpsimd.partition_broadcast(bvbc[:], bv_sb[:])
            wproj_sb = []
            for mt, (m0, msz) in enumerate(NT):
                t = pw.tile([msz, 1024], dt.bfloat16, tag=f"wproj{mt}")
                nc.sync.dma_start(t[:], wproj[m0:m0 + msz, :])
                wproj_sb.append(t)
            wout_sb = pw.tile([128, 4, DIM], dt.bfloat16, tag="wout")
            nc.sync.dma_start(wout_sb[:], wout[:].rearrange("(k p) c -> p k c", p=128))
            tapw_sb = pw.tile([128, 36], dt.float32, tag="tapw")
            nc.sync.dma_start(tapw_sb[:], tapw[:])
            cb_sb = pw.tile([128, 4], dt.float32, tag="cb")
            nc.sync.dma_start(cb_sb[:], cbp[:])
            pb_sb = pw.tile([1, IMG], dt.float32, tag="pb")
            nc.sync.dma_start(pb_sb[:], pbp[:])
            ob_sb = pw.tile([1, DIM], dt.float32, tag="ob")
            nc.sync.dma_start(ob_sb[:], obp[:])
            pbbc = pw.tile([128, IMG], dt.float32, tag="pbbc")
            nc.gpsimd.partition_broadcast(pbbc[:], pb_sb[:])
            obbc = pw.tile([128, DIM], dt.float32, tag="obbc")
            nc.gpsimd.partition_broadcast(obbc[:], ob_sb[:])

            for b in range(BPC):
                # ---- x^T via XBAR transpose DMA ----
                xT = []
                for cb4 in range(4):
                    t = pxT.tile([128, N], dt.bfloat16, tag="xT")
                    c0 = cb4 * 128
                    nc.sync.dma_start_transpose(t[:, 0:1040], xs[b, 0:1040, c0:c0 + 128])
                    nc.sync.dma_start(
                        t[:, 1040:N], xs[b, 1040:N, c0:c0 + 128].rearrange("a b -> b a")
                    )
                    xT.append(t)

                # ---- qk matmul + BN bias, qkvT (h on partitions) ----
                # 6 padded tiles: 0-2 q heads (3/tile @ 0,32,64), 3-5 k heads
                qkvT = []
                for mt6 in range(6):
                    t = pqk.tile([128, N], dt.bfloat16, tag="qkvT")
                    pss = [
                        psmm.tile([128, cw], dt.float32, tag="mm", name=f"qk{c}")
                        for c, (_, cw) in enumerate(NCH)
                    ]
                    for kc in range(4):
                        for c, (ci, cw) in enumerate(NCH):
                            nc.tensor.matmul(
                                pss[c][:],
                                wqkv_sb[:, kc, mt6 * 128:(mt6 + 1) * 128],
                                xT[kc][:, ci:ci + cw],
                                start=(kc == 0), stop=(kc == 3),
                            )
                    for c, (ci, cw) in enumerate(NCH):
                        nc.vector.tensor_scalar(
                            t[:, ci:ci + cw], pss[c][:], bqkv_sb[:, mt6:mt6 + 1], None,
                            op0=alu.add,
                        )
                    qkvT.append(t)

                # ---- v matmul directly in (n, 8*65) layout w/ ones channels ----
                vnat = []
                for nt, (n0, nsz) in enumerate(NT):
                    t = pvn.tile([nsz, 520], dt.bfloat16, tag="vnat")
                    pss = [
                        psmm.tile([nsz, 260], dt.float32, tag="mm", name=f"vn{c}")
                        for c in range(2)
                    ]
                    for kc in range(4):
                        for c in range(2):
                            nc.tensor.matmul(
                                pss[c][:],
                                xT[kc][:, n0:n0 + nsz],
                                wqkv_sb[:, kc, 768 + c * 260:768 + (c + 1) * 260],
                                start=(kc == 0), stop=(kc == 3),
                            )
                    for c in range(2):
                        nc.vector.tensor_tensor(
                            t[:, c * 260:(c + 1) * 260], pss[c][:],
                            bvbc[0:nsz, c * 260:(c + 1) * 260], op=alu.add,
                        )
                    vnat.append(t)

                # ---- stage v for conv reflow ----
                for nt, (n0, nsz) in enumerate(NT):
                    for hh in range(H):
                        nc.sync.dma_start(
                            v_dram[b, hh, n0:n0 + nsz, :],
                            vnat[nt][:, hh * 65:hh * 65 + D],
                        )

                # ---- per head attention ----
                vc_tiles = []
                for hh in range(H):
                    # q/k at base partitions {0,32,64} in padded tiles
                    qo = (hh % 3) * KD
                    qT = qkvT[hh // 3][qo:qo + KD, :]
                    kT = qkvT[3 + hh // 3][qo:qo + KD, :]

                    # attention: scoresT/expT (m part, n free), oT accum
                    oT_ps = [
                        psot.tile([D + 1, cw], dt.float32, tag="ot", name=f"ot{k}")
                        for k, (_, cw) in enumerate(NCH)
                    ]
                    for mt, (m0, msz) in enumerate(NT):
                        et = pexp.tile([128, N], dt.bfloat16, tag="exp")
                        for (ci, cw) in NCH:
                            sc = psmm.tile([msz, cw], dt.float32, tag="mm")
                            nc.tensor.matmul(
                                sc[:], kT[:, m0:m0 + msz], qT[:, ci:ci + cw],
                                start=True, stop=True,
                            )
                            nc.scalar.activation(et[0:msz, ci:ci + cw], sc[:], act_exp)
                        for k, (ci, cw) in enumerate(NCH):
                            nc.tensor.matmul(
                                oT_ps[k][:],
                                vnat[mt][:, hh * 65:hh * 65 + D + 1],
                                et[0:msz, ci:ci + cw],
                                start=(mt == 0), stop=(mt == 8),
                            )
                    oT_sb = poTs.tile([D + 1, N], dt.bfloat16, tag="oTs")
                    for k, (ci, cw) in enumerate(NCH):
                        nc.vector.tensor_copy(oT_sb[:, ci:ci + cw], oT_ps[k][:])
                    # transpose back, normalize, stage o
                    for nt, (n0, nsz) in enumerate(NT):
                        tp2 = psmm.tile([nsz, D + 1], dt.bfloat16, tag="mm")
                        nc.tensor.transpose(
                            tp2[:], oT_sb[:, n0:n0 + nsz], id_sb[0:D + 1, 0:D + 1]
                        )
                        rcp = psm.tile([nsz, 1], dt.float32, tag="rcp")
                        nc.vector.reciprocal(rcp[:], tp2[:, D:D + 1])
                        onrm = psm.tile([nsz, D], dt.bfloat16, tag="onrm")
                        nc.vector.tensor_scalar(
                            onrm[:], tp2[:, 0:D], rcp[:], None, op0=alu.mult
                        )
                        nc.sync.dma_start(
                            o_nat[b, n0:n0 + nsz, hh * D:(hh + 1) * D], onrm[:]
                        )

                    # conv branch per channel-tile once its two heads are staged
                    if hh % 2 == 1:
                        ct = hh // 2
                        cin = pcin.tile([128, 1024], dt.bfloat16, tag="cin")
                        for hl in range(2):
                            src = v_dram[b, 2 * ct + hl, 0:1024, :].rearrange(
                                "(c n2) d -> c (n2 d)", c=64
                            )
                            nc.sync.dma_start(cin[hl * 64:(hl + 1) * 64, :], src)
                        u = pcv.tile([128, 1024], dt.float32, tag="u")
                        nc.vector.tensor_scalar(
                            u[:], cin[:], 1.0 / 6.0, 0.5, op0=alu.mult, op1=alu.add
                        )
                        nc.vector.tensor_scalar(
                            u[:], u[:], 1.0, 0.0, op0=alu.min, op1=alu.max
                        )
                        hs = pcv.tile([128, 32, 32], dt.bfloat16, tag="hs")
                        nc.vector.tensor_tensor(
                            hs[:].rearrange("p a b -> p (a b)"), u[:], cin[:],
                            op=alu.mult,
                        )
                        pad = pcv.tile([128, 34, 34], dt.bfloat16, tag="pad")
                        nc.vector.memset(pad[:], 0.0)
                        nc.vector.tensor_copy(pad[:, 1:33, 1:33], hs[:])
                        acc = pcv.tile([128, 1024], dt.float32, tag="acc")
                        t2 = pcv.tile([128, 1024], dt.float32, tag="t2")
                        for tap in range(9):
                            dy, dx = tap // 3, tap % 3
                            view = pad[:, dy:dy + 32, dx:dx + 32]
                            wsl = tapw_sb[:, ct * 9 + tap:ct * 9 + tap + 1]
                            if tap == 0:
                                nc.vector.tensor_scalar(
                                    acc[:].rearrange("p (a b) -> p a b", a=32),
                                    view, wsl, None, op0=alu.mult,
                                )
                            else:
                                nc.vector.tensor_scalar(
                                    t2[:].rearrange("p (a b) -> p a b", a=32),
                                    view, wsl, None, op0=alu.mult,
                                )
                                nc.vector.tensor_tensor(acc[:], acc[:], t2[:], op=alu.add)
                        nc.vector.tensor_scalar(
                            acc[:], acc[:], cb_sb[:, ct:ct + 1], None, op0=alu.add
                        )
                        nc.vector.tensor_tensor(acc[:], acc[:], pbbc[:], op=alu.add)
                        vct = pvc.tile([128, 1024], dt.bfloat16, tag="vc")
                        nc.vector.tensor_copy(vct[:], acc[:])
                        vc_tiles.append(vct)

                # ---- proj: O2^T tiles via XBAR transpose, xo = O2 @ wproj + vc ----
                O2v = o_nat[b].rearrange("n c -> (n c)").rearrange("(r m) -> r m", m=N)
                o2t = []
                for mt, (m0, msz) in enumerate(NT):
                    t = po2.tile([msz, DH], dt.bfloat16, tag=f"o2t{mt}")
                    if msz == 128:
                        nc.sync.dma_start_transpose(t[:], O2v[:, m0:m0 + msz])
                    else:
                        nc.sync.dma_start(
                            t[:], O2v[:, m0:m0 + msz].rearrange("a b -> b a")
                        )
                    o2t.append(t)
                xo_sb = [
                    pxo.tile([128, 1024], dt.bfloat16, tag="xo", name=f"xo{rt}")
                    for rt in range(4)
                ]
                for rt in range(4):
                    pss = [
                        psmm.tile([128, 512], dt.float32, tag="mm", name=f"xop{c}")
                        for c in range(2)
                    ]
                    for mt, (m0, msz) in enumerate(NT):
                        for c, jc in enumerate((0, 512)):
                            nc.tensor.matmul(
                                pss[c][:],
                                o2t[mt][0:msz, rt * 128:(rt + 1) * 128],
                                wproj_sb[mt][0:msz, jc:jc + 512],
                                start=(mt == 0), stop=(mt == 8),
                            )
                    for c, jc in enumerate((0, 512)):
                        nc.vector.tensor_tensor(
                            xo_sb[rt][:, jc:jc + 512], pss[c][:],
                            vc_tiles[rt][:, jc:jc + 512], op=alu.add,
                        )

                # ---- out = xo^T @ wout^T + out_b ----
                for jt in range(8):
                    ps = psmm.tile([128, 512], dt.float32, tag="mm")
                    for rt in range(4):
                        nc.tensor.matmul(
                            ps[:],
                            xo_sb[rt][:, jt * 128:(jt + 1) * 128],
                            wout_sb[:, rt, :],
                            start=(rt == 0), stop=(rt == 3),
                        )
                    osb = psm.tile([128, DIM], dt.float32, tag="outsb")
                    nc.vector.tensor_tensor(osb[:], ps[:], obbc[:], op=alu.add)
                    nc.sync.dma_start(out_ext[b, jt * 128:(jt + 1) * 128, :], osb[:])

    nc.compile()
    return nc


def _prep_weights(qkv_w, bn_gamma, bn_beta, bn_mean, bn_var,
                  conv_w, conv_b, proj_w, proj_b, out_w, out_b):
    s = bn_gamma / np.sqrt(bn_var + EPS)
    bias = bn_beta - bn_mean * s
    Wt = (qkv_w * s[:, None]).T.copy()
    bvec = bias.copy()
    scale = KD ** -0.5
    for hh in range(H):
        Wt[:, hh * 128:hh * 128 + KD] *= scale
        bvec[hh * 128:hh * 128 + KD] *= scale
    # scatter channels into padded 1288 layout (see _build head slicing):
    # cols 0:768 q/k tiles (3 heads/tile at 0,32,64); 768:1288 v channels
    # hh*65+j with a ones channel (zero weight, bias 1) at j=64
    Wn = np.zeros((DIM, 1288), Wt.dtype)
    bn = np.zeros(1288, bvec.dtype)
    for hh in range(H):
        qdst = (hh // 3) * 128 + (hh % 3) * 32
        kdst = 384 + (hh // 3) * 128 + (hh % 3) * 32
        vdst = 768 + hh * 65
        qsrc, ksrc, vsrc = hh * 128, hh * 128 + 32, hh * 128 + 64
        Wn[:, qdst:qdst + 32] = Wt[:, qsrc:qsrc + 32]
        bn[qdst:qdst + 32] = bvec[qsrc:qsrc + 32]
        Wn[:, kdst:kdst + 32] = Wt[:, ksrc:ksrc + 32]
        bn[kdst:kdst + 32] = bvec[ksrc:ksrc + 32]
        Wn[:, vdst:vdst + 64] = Wt[:, vsrc:vsrc + 64]
        bn[vdst:vdst + 64] = bvec[vsrc:vsrc + 64]
        bn[vdst + 64] = 1.0
    return {
        "wqkv": np.ascontiguousarray(Wn).astype(BF16),
        "bqkv": np.ascontiguousarray(bn[:768].reshape(6, 128).T).astype(np.float32),
        "bv": np.ascontiguousarray(bn[768:].reshape(1, 520)).astype(np.float32),
        "wproj": np.ascontiguousarray(proj_w.T).astype(BF16),
        "wout": np.ascontiguousarray(out_w.T).astype(BF16),
        "tapw": np.ascontiguousarray(
            conv_w[:, 0].reshape(4, 128, 9).transpose(1, 0, 2).reshape(128, 36)
        ).astype(np.float32),
        "cbp": np.ascontiguousarray(conv_b.reshape(4, 128).T).astype(np.float32),
        "pbp": proj_b.reshape(1, IMG).astype(np.float32),
        "obp": out_b.reshape(1, DIM).astype(np.float32),
    }


def run(trace=False, tmpdir=None, **inputs):
    from concourse.bass_utils import run_bass_kernel_spmd

    if "nc" not in _cached:
        _cached["nc"] = _build()
    nc = _cached["nc"]

    w = _prep_weights(**{k: np.asarray(v) for k, v in inputs.items() if k != "x"})
    x = np.asarray(inputs["x"]).astype(BF16)
    in_maps = []
    for c in range(NCORES):
        m = dict(w)
        m["xs"] = np.ascontiguousarray(x[c * BPC:(c + 1) * BPC])
        in_maps.append(m)
    res = run_bass_kernel_spmd(
        nc, in_maps, core_ids=list(range(NCORES)), trace=trace, tmpdir=tmpdir
    )
    out = np.concatenate([np.asarray(r["out"]) for r in res.results], axis=0)
    return out.astype(np.float32), res.exec_time_ns


def kernel(**inputs):
    out, _ = run(trace=False, **inputs)
    return out


if __name__ == "__main__":
    print("building graph...")
    nc = _build()
    print("build OK:", len(nc.m.functions[0].allocations), "allocations")
